# revision 17
# baseline (speedup 1.0000x reference)
"""Chamfer loss on 8 Trainium2 NeuronCores.

Sharding: data-parallel over the batch dim — core b handles batch element b
(one 4096x4096 distance problem per core), emits a single partial sum
S_b = sum_n min_m dist + sum_m min_n dist; the host combines the 8 scalars.

Per-core algorithm:
  d2[n,m] = ||x_n||^2 + ||y_m||^2 - 2 x_n.y_m  is produced by ONE K=13 fp16
  matmul per tile: the x.y term is computed in split precision
  (x ~ xh + xl, y ~ yh + yl in fp16; xy ~ xh*yh + xh*yl + xl*yh) which keeps
  the d2 error ~1e-6 while running at the PE's fast 1-cycle/row rate.
  ||y||^2 and ||x||^2 both ride in the matmul as fp16 hi/lo pairs, so PSUM
  holds the COMPLETE d2 — no ACT bias needed, which lets some tiles skip the
  ACT readout entirely.

  Readout: ACT converts each fp32 PSUM tile to fp16 SBUF with relu (1
  elem/cyc @1.2GHz, ~109us/core) while DVE runs the two min passes (col +
  row, fp16 2x mode @0.96GHz, ~137us/core) — DVE is the only min-capable
  engine on TRN2 (gpsimd has no TensorTensor opcode, DMA-CCE only supports
  add, ACT/DVE accumulators are add-only, fp16 PSUM matmul output is
  TRN3+), so the kernel sits at the exact-all-pairs DVE roofline.
  Row minima: fp16 tensor_tensor min tree on DVE (2x mode).
  Col minima: elementwise min into colacc, then PE-transpose + reduce_min.
  Mins are clamped >= 0 once at the end before sqrt (D-path tiles, if any
  are scheduled, skip the relu).
  sqrt+sum fused via ACT accum_out; partition sum via a ones-matmul.

build_nc(reps=R) unrolls the whole per-core computation R times;
build_nc(loop_reps=R) wraps it in a hardware For_i loop instead (same NEFF
size for any R — used by the timing harness).
"""

import os
import sys

import numpy as np

for _p in ("/opt/trn_rl_repo",):
    if _p not in sys.path and os.path.isdir(_p):
        sys.path.insert(0, _p)

B = 8          # batch (== number of cores)
N = 4096       # points per set
D = 3          # point dim
P = 128        # partitions
MCHUNK = 2048  # m processed per PSUM mega-tile (4 banks)
MM_N = 512     # matmul moving free dim (one PSUM bank)
K = 13         # contraction: xh(3) xh(3) xl(3) 1 1 x2h x2l


def build_nc(n=N, mchunk=MCHUNK, reps=1, loop_reps=0, skip=()):
    """Build the single-core Bass program (SPMD across 8 cores).

    skip: dev-only ablation switches for timing breakdowns
          (subset of {"mm", "act", "col", "row"}).  Skipping breaks
          numerics; only the full build is used for real runs.
    """
    skip = frozenset(skip)
    import concourse.mybir as mybir
    import concourse.tile as tile
    from concourse import bacc
    from concourse.masks import make_identity

    dt = mybir.dt
    Alu = mybir.AluOpType
    Act = mybir.ActivationFunctionType
    AX = mybir.AxisListType

    nt = n // P
    njc = n // mchunk
    banks = mchunk // MM_N

    nc = bacc.Bacc("TRN2", target_bir_lowering=False, debug=False)

    x_dram = nc.dram_tensor("x", [n, D], dt.float32, kind="ExternalInput")
    y_dram = nc.dram_tensor("y", [n, D], dt.float32, kind="ExternalInput")
    out_dram = nc.dram_tensor("out", [1, 1], dt.float32, kind="ExternalOutput")

    with tile.TileContext(nc) as tc:
        with tc.tile_pool(name="singles", bufs=1) as singles:
            ident = singles.tile([P, P], dt.float16)
            make_identity(nc, ident)
            ones_f32 = singles.tile([P, 1], dt.float32)
            nc.gpsimd.memset(ones_f32, 1.0)

            x_nat = singles.tile([P, nt, D], dt.float32)
            y_nat = singles.tile([P, nt, D], dt.float32)
            natX = singles.tile([P, nt, K], dt.float16)
            natY = singles.tile([P, nt, K], dt.float16)
            x2cols = singles.tile([P, nt], dt.float32)
            y2_f32 = singles.tile([P, nt], dt.float32)
            sq_scr = singles.tile([P, nt, D], dt.float32)
            yh_tmp = singles.tile([P, nt, D], dt.float16)
            Xaug = singles.tile([K, n], dt.float16)
            Yaug = singles.tile([K, n], dt.float16)
            colacc = singles.tile([P, njc, mchunk], dt.float16)
            rowmins = singles.tile([P, nt], dt.float32)
            colmins = singles.tile([P, nt], dt.float32)
            sqs = singles.tile([P, nt], dt.float32)
            sqs2 = singles.tile([P, nt], dt.float32)
            rowsum = singles.tile([P, 1], dt.float32)
            colsum = singles.tile([P, 1], dt.float32)
            total = singles.tile([P, 1], dt.float32)
            res_sb = singles.tile([1, 1], dt.float32)

            def emit_iteration(tag):
                # ---------- load + natural-layout aug ----------
                # point index n = p*nt + t (p outer) -> contiguous DMA
                nc.sync.dma_start(
                    out=x_nat, in_=x_dram.ap().rearrange("(p t) d -> p t d", t=nt)
                )
                nc.sync.dma_start(
                    out=y_nat, in_=y_dram.ap().rearrange("(p t) d -> p t d", t=nt)
                )

                # X: [xh xh xl 1 1 x2h x2l]
                nc.gpsimd.memset(natX, 1.0)  # cols 9,10 stay 1
                nc.scalar.copy(out=natX[:, :, 0:3], in_=x_nat)     # xh = f16(x)
                nc.vector.tensor_copy(out=natX[:, :, 3:6], in_=natX[:, :, 0:3])
                nc.vector.tensor_tensor(                            # xl = f16(x-xh)
                    out=natX[:, :, 6:9], in0=x_nat, in1=natX[:, :, 0:3],
                    op=Alu.subtract,
                )
                nc.vector.tensor_mul(sq_scr, x_nat, x_nat)
                nc.vector.tensor_reduce(out=x2cols, in_=sq_scr, axis=AX.X,
                                        op=Alu.add)
                nc.scalar.copy(out=natX[:, :, 11:12], in_=x2cols)   # x2h
                nc.vector.tensor_tensor(                            # x2l
                    out=natX[:, :, 12:13], in0=x2cols, in1=natX[:, :, 11:12],
                    op=Alu.subtract,
                )

                # Y: [-2yh -2yl -2yh y2h y2l 1 1]
                nc.gpsimd.memset(natY, 1.0)  # cols 11,12 stay 1
                nc.scalar.copy(out=yh_tmp, in_=y_nat)               # yh = f16(y)
                nc.scalar.mul(out=natY[:, :, 0:3], in_=yh_tmp, mul=-2.0)
                nc.vector.tensor_copy(out=natY[:, :, 6:9], in_=natY[:, :, 0:3])
                nc.vector.scalar_tensor_tensor(                     # -2yl
                    out=natY[:, :, 3:6], in0=y_nat, scalar=-2.0,
                    in1=natY[:, :, 0:3], op0=Alu.mult, op1=Alu.subtract,
                )
                nc.vector.tensor_mul(sq_scr, y_nat, y_nat)
                nc.vector.tensor_reduce(out=y2_f32, in_=sq_scr, axis=AX.X,
                                        op=Alu.add)
                nc.scalar.copy(out=natY[:, :, 9:10], in_=y2_f32)    # y2h
                nc.vector.tensor_tensor(                            # y2l
                    out=natY[:, :, 10:11], in0=y2_f32, in1=natY[:, :, 9:10],
                    op=Alu.subtract,
                )

                # ---------- transpose to K-major ----------
                with tc.tile_pool(name=f"pp{tag}", bufs=2, space="PSUM") as pp:
                    for (nat, aug) in ((natX, Xaug), (natY, Yaug)):
                        for g in range(nt // 4):
                            ps = pp.tile([K, 4 * P], dt.float16, tag="tp")
                            for q in range(4):
                                nc.tensor.transpose(
                                    ps[:, q * P:(q + 1) * P],
                                    nat[:, g * 4 + q, :], ident,
                                )
                            nc.vector.tensor_copy(
                                out=aug[:, g * 4 * P:(g + 1) * 4 * P], in_=ps
                            )

                if skip:
                    nc.gpsimd.memset(colacc, 60000.0)
                    nc.gpsimd.memset(rowmins, 1.0)
                    nc.gpsimd.memset(colmins, 1.0)

                # ---------- main loop ----------
                with (
                    tc.tile_pool(name=f"pm{tag}", bufs=2, space="PSUM") as pm,
                    tc.tile_pool(name=f"hp{tag}", bufs=4) as hp,
                    tc.tile_pool(name=f"rp{tag}", bufs=2) as rp,
                ):
                    for i in range(nt):
                        h = hp.tile([P, njc, mchunk], dt.float16, tag="h")
                        for jc in range(njc):
                            ps = pm.tile([P, mchunk], dt.float32, tag="d2")
                            if "mm" not in skip:
                                for q in range(banks):
                                    m0 = jc * mchunk + q * MM_N
                                    nc.tensor.matmul(
                                        ps[:, q * MM_N:(q + 1) * MM_N],
                                        lhsT=Xaug[:, i * P:(i + 1) * P],
                                        rhs=Yaug[:, m0:m0 + MM_N],
                                        start=True, stop=True,
                                    )
                            if "act" in skip:
                                continue
                            nc.scalar.activation(
                                out=h[:, jc, :], in_=ps, func=Act.Relu,
                                scale=1.0,
                            )
                        if "act" in skip:
                            continue
                        # one wide col-min over both chunks (fp16 2x mode)
                        if "col" not in skip:
                            if i == 0:
                                nc.vector.tensor_copy(out=colacc, in_=h)
                            else:
                                nc.vector.tensor_tensor(
                                    out=colacc, in0=colacc, in1=h, op=Alu.min,
                                )
                        # row-min: combine the two chunks, then fp16 min tree
                        # (tensor_tensor_reduce would do this in one
                        # instruction but is NRT_EXEC_UNIT_UNRECOVERABLE on
                        # this runtime)
                        if "row" not in skip:
                            rowelem = rp.tile([P, mchunk], dt.float16,
                                              tag="re")
                            if njc > 1:
                                nc.vector.tensor_tensor(
                                    out=rowelem, in0=h[:, 0, :], in1=h[:, 1, :],
                                    op=Alu.min,
                                )
                            else:
                                nc.vector.tensor_copy(out=rowelem,
                                                      in_=h[:, 0, :])
                            w = mchunk // 2
                            while w >= 64:
                                nc.vector.tensor_tensor(
                                    out=rowelem[:, 0:w], in0=rowelem[:, 0:w],
                                    in1=rowelem[:, w:2 * w], op=Alu.min,
                                )
                                w //= 2
                            nc.vector.tensor_reduce(
                                out=rowmins[:, i:i + 1], in_=rowelem[:, 0:64],
                                axis=AX.X, op=Alu.min,
                            )

                # ---------- column partition-reduction ----------
                ngroups = 0 if ("col" in skip or "act" in skip) \
                    else (njc * mchunk) // (8 * P)
                with tc.tile_pool(name=f"pe{tag}", bufs=2, space="PSUM") as pep:
                    for g in range(ngroups):
                        pst = pep.tile([P, 8, P], dt.float16, tag="ct")
                        for k in range(8):
                            base = g * 8 * P + k * P
                            jc, off = divmod(base, mchunk)
                            nc.tensor.transpose(
                                pst[:, k, :], colacc[:, jc, off:off + P], ident
                            )
                        nc.vector.tensor_reduce(
                            out=colmins[:, g * 8:(g + 1) * 8], in_=pst,
                            axis=AX.X, op=Alu.min,
                        )

                    # ---------- clamp, sqrt, sums, partition sum ----------
                    # D-path tiles skip relu; clamp tiny negatives before sqrt
                    nc.vector.tensor_scalar_max(out=rowmins, in0=rowmins,
                                                scalar1=0.0)
                    nc.vector.tensor_scalar_max(out=colmins, in0=colmins,
                                                scalar1=0.0)
                    nc.scalar.activation(
                        out=sqs, in_=rowmins, func=Act.Sqrt, accum_out=rowsum
                    )
                    nc.scalar.activation(
                        out=sqs2, in_=colmins, func=Act.Sqrt, accum_out=colsum
                    )
                    nc.vector.tensor_add(total, rowsum, colsum)
                    ps_sum = pep.tile([1, 1], dt.float32, tag="pssum")
                    nc.tensor.matmul(
                        ps_sum, lhsT=total, rhs=ones_f32, start=True, stop=True
                    )
                    nc.scalar.copy(out=res_sb, in_=ps_sum)
                    nc.sync.dma_start(out=out_dram.ap(), in_=res_sb)

            if loop_reps:
                with tc.For_i(0, loop_reps, 1):
                    emit_iteration("L")
            else:
                for rep in range(reps):
                    emit_iteration(str(rep))
                    if reps > 1:
                        # serialize unrolled reps for standalone-latency timing
                        tc.strict_bb_all_engine_barrier()

    nc.compile()
    return nc


_NC_CACHE = {}


def _get_nc():
    if "nc" not in _NC_CACHE:
        _NC_CACHE["nc"] = build_nc()
    return _NC_CACHE["nc"]


def kernel(set1, set2):
    from concourse import bass_utils

    set1 = np.asarray(set1, dtype=np.float32)
    set2 = np.asarray(set2, dtype=np.float32)
    assert set1.shape == (B, N, D) and set2.shape == (B, N, D)

    nc = _get_nc()
    in_maps = [
        {"x": np.ascontiguousarray(set1[b]), "y": np.ascontiguousarray(set2[b])}
        for b in range(B)
    ]
    res = bass_utils.run_bass_kernel_spmd(nc, in_maps, core_ids=list(range(B)))
    parts = np.array(
        [np.asarray(res.results[b]["out"]).reshape(()) for b in range(B)],
        dtype=np.float64,
    )
    total = parts.sum() / (B * N) / N
    return np.float32(total)
